# revision 11
# baseline (speedup 1.0000x reference)
"""Trainium2 Bass kernel for nn_AttentionMechanisms_1365799600322.

Reference computation (B=4, S=4096, HID=2048, H=16, D=128):
    q = x@Wq+bq; k = x@Wk+bk; v = x@Wv+bv          (reshaped [B,S,H,D])
    scores[b,s,h,g] = q[b,s,h,:]@k[b,s,g,:] * c_scale/sqrt(D)   # per-token HxH
    w = softmax(scores, -1); attn = w@v; out = attn@Wo + bo

Every op mixes only within a token, so we shard the B*S=16384 tokens
across 8 NeuronCores (2048 tokens/core) with zero collectives.

v2 schedule: the per-chunk attention + O-projection work of chunk c-1 is
interleaved into the Q/K/V projection matmul stream of chunk c so the PE
never idles (keeps the 2.4 GHz p-state).  Softmax mask is fused into the
scores matmul as a K=8 additive -50 matmul; normalization is a single
DVE divide (PSUM/PSUM -> bf16); the v [tok,d] -> [(t,g),d] group reshape
is done with PE transposes instead of a DRAM bounce.  The c_scale/sqrt(D)
factor is folded into Wq host-side.  Output is stored bf16 and upcast on
the host.
"""

import numpy as np
import ml_dtypes

import concourse.bass as bass
import concourse.mybir as mybir
from concourse.tile import TileContext

BF16 = mybir.dt.bfloat16
F32 = mybir.dt.float32

B, S, HID, H = 4, 4096, 2048, 16
D = HID // H            # 128
T_TOT = B * S           # 16384
NCORES = 8
T = T_TOT // NCORES     # 2048 tokens per core
KT = HID // 128         # 16 contraction tiles
FT = HID // 128         # 16 feature tiles (== heads for q/k/v layouts)
CHUNK = 512
NCHUNK = T // CHUNK     # 4
GRP = 8                 # tokens per attention group
NGRP = CHUNK // GRP     # 64 groups per chunk
QUAD = 4                # groups per scores/attn psum batch
NBATCH = NGRP // QUAD   # 16
MASK_BIG = 50.0         # additive pre-exp penalty on cross-token slots


def build_nc(with_bias: bool = False):
    nc = bass.Bass()

    # x pre-tiled per chunk: xt[p, (c, kt, t)] = x[tok c*512+t, kt*128+p]
    xt = nc.declare_dram_parameter("xt", [128, NCHUNK * KT * CHUNK], BF16,
                                   isOutput=False)
    # weights pre-tiled: w_h[f*128+p, kt*128+m] = W[kt*128+p, f*128+m]
    wq = nc.declare_dram_parameter("wq", [HID, HID], BF16, isOutput=False)
    wk = nc.declare_dram_parameter("wk", [HID, HID], BF16, isOutput=False)
    wv = nc.declare_dram_parameter("wv", [HID, HID], BF16, isOutput=False)
    wo = nc.declare_dram_parameter("wo", [HID, HID], BF16, isOutput=False)
    # per-partition bias columns: bcols[p, bidx*16+f] = bias_bidx[f*128+p]
    bcols = nc.declare_dram_parameter("bcols", [128, 64], F32, isOutput=False)
    ones = nc.declare_dram_parameter("ones", [128, 128], BF16, isOutput=False)
    ident = nc.declare_dram_parameter("ident", [128, 128], BF16, isOutput=False)
    amask = nc.declare_dram_parameter("amask", [8, 128], BF16, isOutput=False)
    bmask = nc.declare_dram_parameter("bmask", [8, 512], BF16, isOutput=False)
    outT = nc.declare_dram_parameter("outT", [HID, T], BF16, isOutput=True)

    xt_r = xt[:, :].rearrange("p (c n) -> p c n", c=NCHUNK)      # [128,4,8192]
    outT_r = outT[:, :].rearrange("(f p) t -> p f t", p=128)     # [128,16,T]

    with TileContext(nc) as tc:
        from contextlib import ExitStack

        with ExitStack() as _es:
            cpool = _es.enter_context(tc.tile_pool(name="const", bufs=1))
            xpool = _es.enter_context(tc.tile_pool(name="xin", bufs=2))
            wpool = _es.enter_context(tc.tile_pool(name="wstrip", bufs=6))
            qpool = _es.enter_context(tc.tile_pool(name="qq", bufs=2))
            kpool = _es.enter_context(tc.tile_pool(name="kk", bufs=2))
            vtpool = _es.enter_context(tc.tile_pool(name="vt", bufs=1))
            vgpool = _es.enter_context(tc.tile_pool(name="vg", bufs=2))
            apool = _es.enter_context(tc.tile_pool(name="attn", bufs=1))
            ppool = _es.enter_context(tc.tile_pool(name="pexp", bufs=3))
            ripool = _es.enter_context(tc.tile_pool(name="rinv", bufs=2))
            opool = _es.enter_context(tc.tile_pool(name="ostage", bufs=3))
            pp = _es.enter_context(tc.tile_pool(name="pp", bufs=2, space="PSUM"))
            scp = _es.enter_context(tc.tile_pool(name="psc", bufs=2, space="PSUM"))
            rsp = _es.enter_context(tc.tile_pool(name="prs", bufs=2, space="PSUM"))
            atp = _es.enter_context(tc.tile_pool(name="pat", bufs=2, space="PSUM"))

            # ---------------- constants ----------------
            ones_sq = cpool.tile([128, 128], BF16, tag="ones")
            id_sb = cpool.tile([128, 128], BF16, tag="id")
            am_sb = cpool.tile([8, 128], BF16, tag="am")
            bm_sb = cpool.tile([8, 512], BF16, tag="bm")
            bc_sb = cpool.tile([128, 64], F32, tag="bc")
            nc.sync.dma_start(out=ones_sq[:], in_=ones[:, :])
            nc.sync.dma_start(out=id_sb[:], in_=ident[:, :])
            nc.sync.dma_start(out=am_sb[:], in_=amask[:, :])
            nc.sync.dma_start(out=bm_sb[:], in_=bmask[:, :])
            nc.sync.dma_start(out=bc_sb[:], in_=bcols[:, :])

            # Per-chunk SBUF state, created lazily at chunk P-phase start.
            st = [None] * NCHUNK

            def new_chunk_state(c):
                x_sb = xpool.tile([128, KT * CHUNK], BF16, tag="x",
                                  name=f"x{c}")
                qT = qpool.tile([128, H * CHUNK], BF16, tag="qT", name=f"q{c}")
                kT = kpool.tile([128, H * CHUNK], BF16, tag="kT", name=f"k{c}")
                vT = vtpool.tile([128, H * CHUNK], BF16, tag="vT", name=f"v{c}")
                vg = vgpool.tile([128, NGRP * 128], BF16, tag="vg",
                                 name=f"vg{c}")
                at = apool.tile([128, H * CHUNK], BF16, tag="at", name=f"a{c}")
                return dict(x=x_sb, qT=qT, kT=kT, vT=vT, vg=vg, at=at)

            # ---- emission helpers (each is one tensor-stream quantum) ----

            def emit_proj_strip(c, which, f):
                """One projection strip: 16 matmuls + psum->sbuf copy."""
                s = st[c]
                w_h, bidx, dst, eng = {
                    "v": (wv, 2, s["vT"], "v"),
                    "q": (wq, 0, s["qT"], "v"),
                    "k": (wk, 1, s["kT"], "v"),
                }[which]
                w_sb = wpool.tile([128, KT * 128], BF16, tag="w",
                                  name=f"w{which}{c}_{f}")
                nc.sync.dma_start(out=w_sb[:], in_=w_h[f * 128:(f + 1) * 128, :])
                ps = pp.tile([128, CHUNK], F32, tag="pp", name=f"pp{which}{c}_{f}")
                for kt in range(KT):
                    nc.tensor.matmul(
                        ps[:],
                        lhsT=w_sb[:, kt * 128:(kt + 1) * 128],
                        rhs=s["x"][:, kt * CHUNK:(kt + 1) * CHUNK],
                        start=(kt == 0),
                        stop=(kt == KT - 1),
                    )
                # token-interleaved layout: col = t*16 + f
                dst_hm = dst[:].rearrange("p (t h) -> p h t", h=H)
                if with_bias:
                    nc.vector.tensor_scalar_add(
                        dst_hm[:, f, :], ps[:],
                        bc_sb[:, bidx * 16 + f:bidx * 16 + f + 1],
                    )
                else:
                    nc.vector.tensor_copy(out=dst_hm[:, f, :], in_=ps[:])

            def emit_tquad(c, k):
                """Transpose groups 4k..4k+3 of vT into v_grp."""
                s = st[c]
                tp = pp.tile([128, 512], BF16, tag="pp", name=f"tq{c}_{k}")
                for j in range(4):
                    g = 4 * k + j
                    nc.tensor.transpose(
                        tp[:, j * 128:(j + 1) * 128],
                        s["vT"][:, g * 128:(g + 1) * 128],
                        id_sb[:],
                    )
                nc.vector.tensor_copy(
                    out=s["vg"][:, k * 512:(k + 1) * 512], in_=tp[:]
                )

            # attention pipeline state (per source chunk)
            att = {}

            def emit_scores(c, b):
                """Scores for batch b: mask matmul + 4 group matmuls + exp."""
                s = st[c]
                ps = scp.tile([128, 512], F32, tag="sc", name=f"sc{c}_{b}")
                # additive -MASK_BIG on cross-token slots (rank-8 matmul)
                nc.tensor.matmul(
                    ps[:], lhsT=am_sb[:], rhs=bm_sb[:],
                    start=True, stop=False, skip_group_check=True,
                )
                for q in range(QUAD):
                    c0 = (b * QUAD + q) * 128
                    nc.tensor.matmul(
                        ps[:, q * 128:(q + 1) * 128],
                        lhsT=s["kT"][:, c0:c0 + 128],
                        rhs=s["qT"][:, c0:c0 + 128],
                        start=False, stop=True, skip_group_check=True,
                    )
                p_sb = ppool.tile([128, 512], BF16, tag="p", name=f"p{c}_{b}")
                nc.scalar.activation(
                    out=p_sb[:], in_=ps[:],
                    func=mybir.ActivationFunctionType.Exp,
                )
                att[(c, b)] = p_sb

            def emit_attn(c, b):
                """Rowsum + attn matmuls + divide for batch b."""
                s = st[c]
                p_sb = att.pop((c, b))
                prs = rsp.tile([128, 512], F32, tag="rs", name=f"rs{c}_{b}")
                nc.tensor.matmul(prs[:], lhsT=ones_sq, rhs=p_sb[:],
                                 start=True, stop=True)
                pat = atp.tile([128, 512], F32, tag="at", name=f"pa{c}_{b}")
                for q in range(QUAD):
                    g = b * QUAD + q
                    nc.tensor.matmul(
                        pat[:, q * 128:(q + 1) * 128],
                        lhsT=s["vg"][:, g * 128:(g + 1) * 128],
                        rhs=p_sb[:, q * 128:(q + 1) * 128],
                        start=True, stop=True,
                    )
                rinv = ripool.tile([128, 512], F32, tag="ri", name=f"ri{c}_{b}")
                nc.vector.reciprocal(out=rinv[:], in_=prs[:])
                nc.vector.tensor_tensor(
                    out=s["at"][:, b * 512:(b + 1) * 512],
                    in0=pat[:], in1=rinv[:],
                    op=mybir.AluOpType.mult,
                )

            def emit_o_strip(c, f):
                """One O-projection strip + bf16 store."""
                s = st[c]
                tok0 = c * CHUNK
                w_sb = wpool.tile([128, KT * 128], BF16, tag="w",
                                  name=f"wo{c}_{f}")
                nc.sync.dma_start(out=w_sb[:], in_=wo[f * 128:(f + 1) * 128, :])
                ps = pp.tile([128, CHUNK], F32, tag="pp", name=f"ppo{c}_{f}")
                at_hm = s["at"][:].rearrange("p (t h) -> p h t", h=H)
                for kt in range(KT):
                    nc.tensor.matmul(
                        ps[:],
                        lhsT=w_sb[:, kt * 128:(kt + 1) * 128],
                        rhs=at_hm[:, kt, :],
                        start=(kt == 0),
                        stop=(kt == KT - 1),
                    )
                o_sb = opool.tile([128, CHUNK], BF16, tag="o", name=f"o{c}_{f}")
                if with_bias:
                    nc.vector.tensor_scalar_add(
                        o_sb[:], ps[:], bc_sb[:, 48 + f:48 + f + 1]
                    )
                else:
                    nc.vector.tensor_copy(out=o_sb[:], in_=ps[:])
                nc.sync.dma_start(
                    out=outT_r[:, f, tok0:tok0 + CHUNK], in_=o_sb[:]
                )

            # ---------------- master interleaved schedule ----------------
            # chunk-c emission block: P+T phase of chunk c, A+O of chunk c-1.
            for c in range(NCHUNK + 1):
                if c < NCHUNK:
                    st[c] = new_chunk_state(c)
                    nc.sync.dma_start(
                        out=st[c]["x"][:], in_=xt_r[:, c, :]
                    )
                a = c - 1  # attention/O source chunk
                if c < NCHUNK:
                    # positions 0..15: V strips + attention quanta of a
                    for i in range(16):
                        emit_proj_strip(c, "v", i)
                        if a >= 0:
                            if i < NBATCH:
                                emit_scores(a, i)
                            if 1 <= i <= NBATCH:
                                emit_attn(a, i - 1)
                    # positions 16..47: Q,K strips + tquads + O strips of a
                    for i in range(32):
                        which, f = ("q", i) if i < 16 else ("k", i - 16)
                        emit_proj_strip(c, which, f)
                        if i == 0 and a >= 0:
                            emit_attn(a, NBATCH - 1)
                        if i < 16:
                            emit_tquad(c, i)
                        elif a >= 0:
                            emit_o_strip(a, i - 16)
                else:
                    # epilogue: attention + O of the last chunk
                    for b in range(NBATCH):
                        emit_scores(a, b)
                        if b >= 1:
                            emit_attn(a, b - 1)
                    emit_attn(a, NBATCH - 1)
                    for f in range(FT):
                        emit_o_strip(a, f)

    return nc


# Opcodes whose encodings accept multiple sync waits. On TRN2 every TPB
# engine instruction (and the DMA pseudo-instruction) takes at most ONE
# wait, so surplus waits are split into standalone EventSemaphore
# instructions spliced just before the offender (same engine stream =>
# identical semantics).
_WAIT_BUDGET = {}


def _split_waits_json(bir: bytes) -> bytes:
    import orjson

    j = orjson.loads(bir)
    ctr = 0
    for fn in j["functions"]:
        for blk in fn["blocks"]:
            out = []
            for ins in blk["instructions"]:
                si = ins.get("sync_info")
                waits = (si or {}).get("on_wait") or []
                budget = _WAIT_BUDGET.get(ins.get("opcode"), 1)
                if len(waits) > budget:
                    for w in waits[:-budget]:
                        ctr += 1
                        out.append(
                            {
                                "debug": ins.get("debug", 0),
                                "engine": ins["engine"],
                                "ins": [],
                                "name": f"Wsplit-{ctr}",
                                "opcode": "EventSemaphore",
                                "outs": [],
                                "sync_info": {"on_update": [], "on_wait": [w]},
                            }
                        )
                    si["on_wait"] = waits[-budget:]
                out.append(ins)
            blk["instructions"] = out
    return orjson.dumps(j)


def _install_ntff_shim():
    """This image's antenv lacks axon_hooks; provide it so trace=True works."""
    import sys, types

    if "antenv.axon_hooks" in sys.modules:
        return
    mod = types.ModuleType("antenv.axon_hooks")
    mod._hook = None

    def set_axon_ntff_profile_hook(h):
        mod._hook = h

    def get_axon_ntff_profile_hook():
        return mod._hook

    mod.set_axon_ntff_profile_hook = set_axon_ntff_profile_hook
    mod.get_axon_ntff_profile_hook = get_axon_ntff_profile_hook
    sys.modules["antenv.axon_hooks"] = mod
    try:
        import antenv

        antenv.axon_hooks = mod
    except ImportError:
        pass
    try:
        from trn_agent_boot.trn_boot import _ntff_profile_via_ctypes

        mod.set_axon_ntff_profile_hook(
            _ntff_profile_via_ctypes("/opt/axon/libaxon_pjrt.so")
        )
    except Exception as e:  # degrade: tracing skipped, run still works
        print(f"ntff shim: hook registration failed: {e}")


def _host_inputs(x, Wq, bq, Wk, bk, Wv, bv, Wo, bo, c_scale):
    """Build per-core in_maps (host-side shard + tile + bf16 cast)."""
    bf = ml_dtypes.bfloat16
    xf = np.ascontiguousarray(np.asarray(x, np.float32).reshape(T_TOT, HID))
    scale = float(np.asarray(c_scale, np.float32).reshape(-1)[0]) / np.sqrt(D)

    def tile_w(W, s=1.0):  # w_h[f*128+p, kt*128+m] = W[kt*128+p, f*128+m]
        Wb = (np.asarray(W, np.float32) * s).astype(bf)
        return np.ascontiguousarray(
            Wb.reshape(KT, 128, FT, 128).transpose(2, 1, 0, 3).reshape(HID, HID)
        )

    # bcols[p, bidx*16+f] = bias_bidx[f*128+p]  (bq pre-scaled)
    bcat = np.stack([
        np.asarray(bq, np.float32) * scale,
        np.asarray(bk, np.float32),
        np.asarray(bv, np.float32),
        np.asarray(bo, np.float32),
    ])  # [4, 2048]
    bcols_np = np.ascontiguousarray(
        bcat.reshape(4, 16, 128).transpose(2, 0, 1).reshape(128, 64)
    ).astype(np.float32)

    ones_b = np.ones((128, 128), bf)
    ident = np.eye(128, dtype=np.float32).astype(bf)
    # amask[tau, (t,g)] = 1{t==tau}; bmask[tau, (t',h)] = -BIG*1{t'!=tau}
    am = np.zeros((8, 128), np.float32)
    bm = np.zeros((8, 512), np.float32)
    for tau in range(8):
        am[tau, tau * 16:(tau + 1) * 16] = 1.0
        for q in range(QUAD):
            bm[tau, q * 128:(q + 1) * 128] = -MASK_BIG
            bm[tau, q * 128 + tau * 16:q * 128 + (tau + 1) * 16] = 0.0
    am = am.astype(bf)
    bm = bm.astype(bf)

    shared = dict(
        wq=tile_w(Wq, scale), wk=tile_w(Wk), wv=tile_w(Wv), wo=tile_w(Wo),
        bcols=bcols_np, ones=ones_b, ident=ident, amask=am, bmask=bm,
    )
    in_maps = []
    for i in range(NCORES):
        xc = xf[i * T:(i + 1) * T]                       # [T, HID]
        # xt[p, c, kt, t] = x[c*512+t, kt*128+p]
        xt_i = np.ascontiguousarray(
            xc.reshape(NCHUNK, CHUNK, KT, 128)
              .transpose(3, 0, 2, 1)
              .reshape(128, NCHUNK * KT * CHUNK)
              .astype(bf)
        )
        in_maps.append(dict(xt=xt_i, **shared))
    return in_maps


def _assemble(results):
    outs = []
    for i in range(NCORES):
        o = np.asarray(results[i]["outT"]).astype(np.float32).T  # [T, HID]
        outs.append(o)
    return np.concatenate(outs, axis=0).reshape(B, S, HID)


def run(inputs: dict, trace: bool = False):
    """Compile + execute on 8 cores; returns (output, BassKernelResults)."""
    from concourse.bass_utils import run_bass_kernel_spmd

    if trace:
        _install_ntff_shim()
    wb = any(
        np.any(np.asarray(inputs[k], np.float32) != 0.0)
        for k in ("bq", "bk", "bv", "bo")
    )
    nc = build_nc(with_bias=wb)
    _orig_tjb = nc.to_json_bytes
    nc.to_json_bytes = lambda: _split_waits_json(_orig_tjb())
    in_maps = _host_inputs(**inputs)
    res = run_bass_kernel_spmd(
        nc, in_maps, core_ids=list(range(NCORES)), trace=trace
    )
    return _assemble(res.results), res


def kernel(**inputs) -> np.ndarray:
    out, _ = run(inputs, trace=False)
    return out
